# revision 7
# baseline (speedup 1.0000x reference)
import os
import sys
import types

import numpy as np

sys.path.insert(0, "/opt/trn_rl_repo")

import ml_dtypes  # noqa: E402
import concourse.mybir as mybir  # noqa: E402
import concourse.tile as tile  # noqa: E402
from concourse import bacc  # noqa: E402
from concourse.bass import ts  # noqa: E402
from concourse.bass_utils import run_bass_kernel_spmd  # noqa: E402

BF16 = mybir.dt.bfloat16
F32 = mybir.dt.float32
bfdt = ml_dtypes.bfloat16
AF = mybir.ActivationFunctionType
ALU = mybir.AluOpType

B, D, N = 4, 512, 2048
H, KVH, DH = 8, 2, 64
CONTEXT_LEN = 4096
NLOC = 1024  # tokens per core
P = 128
NCORES = 8
HP = H // 2  # head pairs
NCH = N // P  # 16 key chunks of 128
NCH_LOC = NLOC // P  # 8 local key chunks

_CACHE = {}


def _enable_trace_hook():
    """Register the NTFF profile hook (missing antenv.axon_hooks shim)."""
    try:
        import antenv

        if "antenv.axon_hooks" in sys.modules:
            return
        mod = types.ModuleType("antenv.axon_hooks")

        def set_axon_ntff_profile_hook(h):
            mod._hook = h

        def get_axon_ntff_profile_hook():
            return getattr(mod, "_hook", None)

        mod.set_axon_ntff_profile_hook = set_axon_ntff_profile_hook
        mod.get_axon_ntff_profile_hook = get_axon_ntff_profile_hook
        sys.modules["antenv.axon_hooks"] = mod
        antenv.axon_hooks = mod
        from trn_agent_boot.trn_boot import _ntff_profile_via_ctypes

        set_axon_ntff_profile_hook(_ntff_profile_via_ctypes("/opt/axon/libaxon_pjrt.so"))
    except Exception:
        pass


def _build():
    nc = bacc.Bacc(None, target_bir_lowering=False, debug=False)
    dp = nc.declare_dram_parameter

    x_e = dp("x", [4, P, NLOC], F32, isOutput=False)
    wq_e = dp("wq", [P, 4, 512], BF16, isOutput=False)
    wqs_e = dp("wqs", [P, 4, 512], BF16, isOutput=False)
    wk_e = dp("wk", [P, 4, 128], BF16, isOutput=False)
    wks_e = dp("wks", [P, 4, 128], BF16, isOutput=False)
    wv_e = dp("wv", [P, 4, 128], BF16, isOutput=False)
    wo_e = dp("wo", [P, 4, 512], BF16, isOutput=False)
    cq_e = dp("cq", [P, NLOC], F32, isOutput=False)
    sq_e = dp("sq", [P, NLOC], F32, isOutput=False)
    ck_e = dp("ck", [P, NLOC], F32, isOutput=False)
    sk_e = dp("sk", [P, NLOC], F32, isOutput=False)
    g_e = dp("g", [1, 512], F32, isOutput=False)  # gamma row
    gb_e = dp("gb", [2, 512], F32, isOutput=False)  # [gamma; beta]
    bo_e = dp("bo", [P, 4], F32, isOutput=False)  # bout per (p, chunk)
    ones_e = dp("ones", [P, 1], F32, isOutput=False)
    out_e = dp("out", [4, P, NLOC], F32, isOutput=True)

    with tile.TileContext(nc) as tc:
        with (
            tc.tile_pool(name="persist", bufs=1) as PS,
            tc.tile_pool(name="tmp", bufs=3) as TMP,
            tc.tile_pool(name="exp", bufs=4) as EXPP,
            tc.tile_pool(name="dram", bufs=1, space="DRAM") as DRAM,
        ):
            # ---------------- phase A: inputs -> SBUF ----------------
            x_sb = [PS.tile([P, NLOC], F32, name=f"x{c}") for c in range(4)]
            for c in range(4):
                nc.sync.dma_start(x_sb[c][:], x_e[c])
            ones_sb = PS.tile([P, 1], F32, name="ones")
            nc.sync.dma_start(ones_sb[:], ones_e[:])
            g_sb = PS.tile([1, 512], F32, name="g")
            nc.sync.dma_start(g_sb[:], g_e[:])
            gb_sb = PS.tile([2, 512], F32, name="gb")
            nc.sync.dma_start(gb_sb[:], gb_e[:])
            bo_sb = PS.tile([P, 4], F32, name="bo")
            nc.sync.dma_start(bo_sb[:], bo_e[:])
            wk_sb = PS.tile([P, 4, 128], BF16, name="wk")
            nc.sync.dma_start(wk_sb[:], wk_e[:])
            wks_sb = PS.tile([P, 4, 128], BF16, name="wks")
            nc.sync.dma_start(wks_sb[:], wks_e[:])
            wv_sb = PS.tile([P, 4, 128], BF16, name="wv")
            nc.sync.dma_start(wv_sb[:], wv_e[:])
            ck_sb = PS.tile([P, NLOC], F32, name="ck")
            nc.sync.dma_start(ck_sb[:], ck_e[:])
            sk_sb = PS.tile([P, NLOC], F32, name="sk")
            nc.sync.dma_start(sk_sb[:], sk_e[:])
            wq_sb = PS.tile([P, 4, 512], BF16, name="wq")
            nc.sync.dma_start(wq_sb[:], wq_e[:])
            wqs_sb = PS.tile([P, 4, 512], BF16, name="wqs")
            nc.sync.dma_start(wqs_sb[:], wqs_e[:])
            cq_sb = PS.tile([P, NLOC], F32, name="cq")
            nc.sync.dma_start(cq_sb[:], cq_e[:])
            sq_sb = PS.tile([P, NLOC], F32, name="sq")
            nc.sync.dma_start(sq_sb[:], sq_e[:])
            wo_sb = PS.tile([P, 4, 512], BF16, name="wo")
            nc.sync.dma_start(wo_sb[:], wo_e[:])

            # v lhsT store: slot s = 2*chunk + kvh, [tk, dh+1] with ones col
            v_sb = PS.tile([P, 2 * NCH, DH + 1], BF16, name="vsb")
            nc.gpsimd.memset(v_sb[:, :, DH : DH + 1], 1.0)

            xnb = [PS.tile([P, NLOC], BF16, name=f"xnb{c}") for c in range(4)]
            qr_sb = [PS.tile([P, NLOC], BF16, name=f"qr{i}") for i in range(HP)]
            k_bf = PS.tile([P, NLOC], BF16, name="kbf")
            k_rem = PS.tile([P, NLOC], BF16, name="krem")
            vcp_sb = PS.tile([P, NCH_LOC, 128], BF16, name="vcp")
            ohat = [PS.tile([P, NLOC], BF16, name=f"oh{i}") for i in range(HP)]
            rb_sb = PS.tile([2, NLOC], F32, name="rb")  # [mu*rstd; -1]
            nc.gpsimd.memset(rb_sb[:], -1.0)  # row 0 overwritten with mu*rstd below
            rstd_sb = PS.tile([1, NLOC], F32, name="rstd")
            mu_sb = PS.tile([1, NLOC], F32, name="mu")
            ex2_sb = PS.tile([1, NLOC], F32, name="ex2")
            var_sb = PS.tile([1, NLOC], F32, name="var")
            sd_sb = PS.tile([1, NLOC], F32, name="sd")
            musq_sb = PS.tile([1, NLOC], F32, name="musq")

            ag_in = DRAM.tile([2, P, NLOC], BF16)
            ag_out = DRAM.tile([2, 2, P, NLOC], BF16)

            # ---------------- phase B: layernorm ----------------
            with tc.tile_pool(name="ps_b", bufs=1, space="PSUM") as PB:
                stats = PB.tile([1, 4, 512], F32, name="stats")
                for tq in range(2):
                    for c in range(4):
                        nc.tensor.matmul(
                            stats[:, tq, :], ones_sb[:], x_sb[c][:, ts(tq, 512)],
                            start=(c == 0), stop=(c == 3),
                        )
                for c in range(4):
                    xsq = TMP.tile([P, NLOC], F32, tag="xsq")
                    nc.scalar.activation(xsq[:], x_sb[c][:], AF.Square)
                    for tq in range(2):
                        nc.tensor.matmul(
                            stats[:, 2 + tq, :], ones_sb[:], xsq[:, ts(tq, 512)],
                            start=(c == 0), stop=(c == 3),
                        )
                nc.scalar.mul(mu_sb[:], stats[:, 0:2, :].rearrange("p a b -> p (a b)"), 1.0 / 512)
                nc.scalar.activation(
                    ex2_sb[:], stats[:, 2:4, :].rearrange("p a b -> p (a b)"),
                    AF.Copy, bias=1e-5, scale=1.0 / 512,
                )
                nc.vector.tensor_mul(musq_sb[:], mu_sb[:], mu_sb[:])
                nc.vector.tensor_tensor(var_sb[:], ex2_sb[:], musq_sb[:], ALU.subtract)
                nc.scalar.activation(sd_sb[:], var_sb[:], AF.Sqrt)
                nc.vector.reciprocal(rstd_sb[:], sd_sb[:])
                nc.vector.tensor_mul(rb_sb[0:1, :], mu_sb[:], rstd_sb[:])

                # xn = x * (gamma (x) rstd) - (gamma (x) mu*rstd - beta (x) 1)
                for c in range(4):
                    a_ps = PB.tile([P, 2, 512], F32, name="aps")
                    b_ps = PB.tile([P, 2, 512], F32, name="bps")
                    for tq in range(2):
                        nc.tensor.matmul(
                            a_ps[:, tq, :], g_sb[0:1, ts(c, 128)],
                            rstd_sb[:, ts(tq, 512)], start=True, stop=True,
                        )
                        nc.tensor.matmul(
                            b_ps[:, tq, :], gb_sb[:, ts(c, 128)],
                            rb_sb[:, ts(tq, 512)], start=True, stop=True,
                        )
                    t1 = TMP.tile([P, NLOC], F32, tag="t")
                    nc.vector.tensor_mul(
                        t1[:], x_sb[c][:], a_ps[:].rearrange("p a b -> p (a b)")
                    )
                    nc.vector.tensor_tensor(
                        xnb[c][:], t1[:], b_ps[:].rearrange("p a b -> p (a b)"),
                        ALU.subtract,
                    )

            # ---------------- phase C1: k/v projection, rotary, allgather ----------------
            with tc.tile_pool(name="ps_c1", bufs=1, space="PSUM") as PC:
                kp0 = PC.tile([P, 2, 512], F32, name="kp0")
                kp1 = PC.tile([P, 2, 512], F32, name="kp1")
                for sw, (kps, w) in enumerate(((kp0, wk_sb), (kp1, wks_sb))):
                    for tq in range(2):
                        for c in range(4):
                            nc.tensor.matmul(
                                kps[:, tq, :], w[:, c, :], xnb[c][:, ts(tq, 512)],
                                start=(c == 0), stop=(c == 3),
                            )
                t1 = TMP.tile([P, NLOC], F32, tag="t")
                t2 = TMP.tile([P, NLOC], F32, tag="t")
                nc.vector.tensor_mul(t1[:], ck_sb[:], kp0[:].rearrange("p a b -> p (a b)"))
                nc.vector.tensor_mul(t2[:], sk_sb[:], kp1[:].rearrange("p a b -> p (a b)"))
                nc.vector.tensor_add(k_bf[:], t1[:], t2[:])

                for c8 in range(NCH_LOC):
                    vp = PC.tile([P, 128], F32, name=f"vp{c8 % 2}")
                    for c in range(4):
                        nc.tensor.matmul(
                            vp[:], xnb[c][:, ts(c8, 128)], wv_sb[:, c, :],
                            start=(c == 0), stop=(c == 3),
                        )
                    nc.vector.tensor_copy(vcp_sb[:, c8, :], vp[:])

                nc.sync.dma_start(ag_in[0], k_bf[:])
                nc.sync.dma_start(
                    ag_in[1], vcp_sb[:].rearrange("p a b -> p (a b)")
                )
                nc.gpsimd.collective_compute(
                    "AllGather",
                    ALU.bypass,
                    ins=[ag_in[:]],
                    outs=[ag_out[:]],
                    replica_groups=[[0, 1], [2, 3], [4, 5], [6, 7]],
                )

            # local v -> slots 0..15
            nc.vector.tensor_copy(
                v_sb[:, 0 : 2 * NCH_LOC, 0:DH],
                vcp_sb[:].rearrange("p a (g d) -> p (a g) d", g=2),
            )

            # ---------------- phase C2: q projection + rotary ----------------
            with tc.tile_pool(name="ps_c2", bufs=1, space="PSUM") as PC2:
                for i in range(HP):
                    qp0 = PC2.tile([P, 2, 512], F32, name="qp0")
                    qp1 = PC2.tile([P, 2, 512], F32, name="qp1")
                    for kps, w in ((qp0, wq_sb), (qp1, wqs_sb)):
                        for tq in range(2):
                            for c in range(4):
                                nc.tensor.matmul(
                                    kps[:, tq, :], w[:, c, ts(i, 128)],
                                    xnb[c][:, ts(tq, 512)],
                                    start=(c == 0), stop=(c == 3),
                                )
                    t1 = TMP.tile([P, NLOC], F32, tag="t")
                    t2 = TMP.tile([P, NLOC], F32, tag="t")
                    nc.vector.tensor_mul(t1[:], cq_sb[:], qp0[:].rearrange("p a b -> p (a b)"))
                    nc.vector.tensor_mul(t2[:], sq_sb[:], qp1[:].rearrange("p a b -> p (a b)"))
                    nc.vector.tensor_add(qr_sb[i][:], t1[:], t2[:])

            # ---------------- remote kv recovery: remote = (ag0 + ag1) - local ----------------
            agk0 = PS.tile([P, NLOC], BF16, name="agk0")
            agk1 = PS.tile([P, NLOC], BF16, name="agk1")
            agv0 = PS.tile([P, NLOC], BF16, name="agv0")
            agv1 = PS.tile([P, NLOC], BF16, name="agv1")
            nc.sync.dma_start(agk0[:], ag_out[0, 0])
            nc.sync.dma_start(agk1[:], ag_out[1, 0])
            nc.sync.dma_start(agv0[:], ag_out[0, 1])
            nc.sync.dma_start(agv1[:], ag_out[1, 1])
            tk = TMP.tile([P, NLOC], F32, tag="t")
            nc.vector.tensor_add(tk[:], agk0[:], agk1[:])
            nc.vector.tensor_tensor(k_rem[:], tk[:], k_bf[:], ALU.subtract)
            tv = TMP.tile([P, NLOC], F32, tag="t")
            nc.vector.tensor_add(tv[:], agv0[:], agv1[:])
            nc.vector.tensor_tensor(
                v_sb[:, 2 * NCH_LOC : 2 * NCH, 0:DH],
                tv[:].rearrange("p (a g d) -> p (a g) d", g=2, d=DH),
                vcp_sb[:].rearrange("p a (g d) -> p (a g) d", g=2),
                ALU.subtract,
            )

            # ---------------- phase D: attention main loop ----------------
            with (
                tc.tile_pool(name="ps_sc", bufs=2, space="PSUM") as PSC,
                tc.tile_pool(name="ps_av", bufs=2, space="PSUM") as PAV,
            ):
                for hp in range(HP):
                    for tq in range(2):
                        oA = PAV.tile([DH + 1, 512], F32, name="avA")
                        oB = PAV.tile([DH + 1, 512], F32, name="avB")
                        for cp in range(NCH // 2):
                            scA = PSC.tile([P, 2, 512], F32, tag="sc")
                            scB = PSC.tile([P, 2, 512], F32, tag="sc")
                            for j in range(2):
                                c = 2 * cp + j
                                if c < NCH_LOC:
                                    ksrc = k_bf[:, ts(c, 128)]
                                else:
                                    ksrc = k_rem[:, ts(c - NCH_LOC, 128)]
                                nc.tensor.matmul(
                                    scA[:, j, :], ksrc[0:64, :],
                                    qr_sb[hp][0:64, ts(tq, 512)],
                                    start=True, stop=True,
                                )
                                nc.tensor.matmul(
                                    scB[:, j, :], ksrc[64:128, :],
                                    qr_sb[hp][64:128, ts(tq, 512)],
                                    start=True, stop=True,
                                )
                            eA = EXPP.tile([P, NLOC], BF16, tag="e")
                            eB = EXPP.tile([P, NLOC], BF16, tag="e")
                            nc.scalar.activation(
                                eA[:], scA[:].rearrange("p a b -> p (a b)"),
                                AF.Exp, scale=0.125,
                            )
                            nc.scalar.activation(
                                eB[:], scB[:].rearrange("p a b -> p (a b)"),
                                AF.Exp, scale=0.125,
                            )
                            for j in range(2):
                                c = 2 * cp + j
                                nc.tensor.matmul(
                                    oA[:], v_sb[:, 2 * c + 0, :], eA[:, ts(j, 512)],
                                    start=(cp == 0 and j == 0),
                                    stop=(cp == NCH // 2 - 1 and j == 1),
                                )
                                nc.tensor.matmul(
                                    oB[:], v_sb[:, 2 * c + 1, :], eB[:, ts(j, 512)],
                                    start=(cp == 0 and j == 0),
                                    stop=(cp == NCH // 2 - 1 and j == 1),
                                )
                        # epilogue: divide by denominator row, write o_hat
                        den2 = TMP.tile([1, 1024], F32, tag="den")
                        nc.vector.tensor_copy(den2[0:1, 0:512], oA[DH : DH + 1, :])
                        nc.vector.tensor_copy(den2[0:1, 512:1024], oB[DH : DH + 1, :])
                        rec2 = TMP.tile([1, 1024], F32, tag="den")
                        nc.vector.reciprocal(rec2[:], den2[:])
                        pbA = TMP.tile([64, 512], F32, tag="pb")
                        pbB = TMP.tile([64, 512], F32, tag="pb")
                        nc.gpsimd.partition_broadcast(pbA[:], rec2[0:1, 0:512])
                        nc.gpsimd.partition_broadcast(pbB[:], rec2[0:1, 512:1024])
                        nc.vector.tensor_mul(
                            ohat[hp][0:64, ts(tq, 512)], oA[0:DH, :], pbA[:]
                        )
                        nc.vector.tensor_mul(
                            ohat[hp][64:128, ts(tq, 512)], oB[0:DH, :], pbB[:]
                        )

            # ---------------- phase E: output projection + residual ----------------
            with tc.tile_pool(name="ps_e", bufs=4, space="PSUM") as PE_:
                for mc in range(4):
                    for tq in range(2):
                        yps = PE_.tile([P, 512], F32, tag="yps")
                        for kc in range(4):
                            nc.tensor.matmul(
                                yps[:], wo_sb[:, kc, ts(mc, 128)],
                                ohat[kc][:, ts(tq, 512)],
                                start=(kc == 0), stop=(kc == 3),
                            )
                        yt = TMP.tile([P, 512], F32, tag="yout")
                        nc.vector.tensor_add(yt[:], yps[:], xnb[mc][:, ts(tq, 512)])
                        yo = TMP.tile([P, 512], F32, tag="yout")
                        nc.vector.tensor_scalar_add(yo[:], yt[:], bo_sb[:, mc : mc + 1])
                        nc.sync.dma_start(out_e[mc, :, ts(tq, 512)], yo[:])

    nc.compile()
    return nc


def _host_inputs(x, gamma, beta, Wq, Wkv, Wout, bout):
    """Build the 8 per-core input maps."""
    x = np.asarray(x, np.float32)
    gamma = np.asarray(gamma, np.float32)
    beta = np.asarray(beta, np.float32)
    Wq = np.asarray(Wq, np.float32)
    Wkv = np.asarray(Wkv, np.float32)
    Wout = np.asarray(Wout, np.float32)
    bout = np.asarray(bout, np.float32)

    def swap_heads(W):
        # permute output cols j -> j xor 32 within each 64-block
        c = W.shape[1]
        return np.ascontiguousarray(
            W.reshape(D, c // 64, 2, 32)[:, :, ::-1, :].reshape(D, c)
        )

    def lhsT(W):
        # [D, M] -> [128, 4, M] chunk layout
        return np.ascontiguousarray(
            W.reshape(4, P, W.shape[1]).transpose(1, 0, 2).astype(bfdt)
        )

    Wk = Wkv[:, : KVH * DH]
    Wv = Wkv[:, KVH * DH :]
    wq = lhsT(Wq)
    wqs = lhsT(swap_heads(Wq))
    wk = lhsT(Wk)
    wks = lhsT(swap_heads(Wk))
    wv = lhsT(Wv)
    wo = lhsT(Wout)
    g = np.ascontiguousarray(gamma[None, :])
    gb = np.ascontiguousarray(np.stack([gamma, beta]))
    bo = np.ascontiguousarray(bout.reshape(4, P).T)
    ones = np.ones((P, 1), np.float32)

    # rotary tables (per half)
    j = np.arange(DH)
    inv_freq = 1.0 / (10000.0 ** ((2.0 * (j % 32)) / DH))
    base = ((2.0 * (j % 32)) + 0.4 * DH) / (1.4 * DH)
    sign = np.where(j < 32, -1.0, 1.0)

    tables = []
    for half in range(2):
        pos = half * NLOC + np.arange(NLOC, dtype=np.float64)
        freqs = pos[None, :] * inv_freq[:, None]  # [64, NLOC]
        cos, sin = np.cos(freqs), np.sin(freqs)
        power = (pos - N // 2) / CONTEXT_LEN
        xsc = base[:, None] ** power[None, :]
        cq = np.tile((cos * xsc), (2, 1)).astype(np.float32)
        sq = np.tile((sign[:, None] * sin * xsc), (2, 1)).astype(np.float32)
        ck = np.tile((cos / xsc), (2, 1)).astype(np.float32)
        sk = np.tile((sign[:, None] * sin / xsc), (2, 1)).astype(np.float32)
        tables.append((cq, sq, ck, sk))

    in_maps = []
    for core in range(NCORES):
        b, half = core // 2, core % 2
        xc = np.ascontiguousarray(
            x[b].reshape(4, P, N)[:, :, half * NLOC : (half + 1) * NLOC]
        )
        cq, sq, ck, sk = tables[half]
        in_maps.append(
            {
                "x": xc, "wq": wq, "wqs": wqs, "wk": wk, "wks": wks,
                "wv": wv, "wo": wo, "cq": cq, "sq": sq, "ck": ck, "sk": sk,
                "g": g, "gb": gb, "bo": bo, "ones": ones,
            }
        )
    return in_maps


def kernel(x, gamma, beta, Wq, Wkv, Wout, bout):
    trace = os.environ.get("KERNEL_TRACE", "0") == "1"
    if trace:
        _enable_trace_hook()
    if "nc" not in _CACHE:
        _CACHE["nc"] = _build()
    nc = _CACHE["nc"]
    in_maps = _host_inputs(x, gamma, beta, Wq, Wkv, Wout, bout)
    res = run_bass_kernel_spmd(nc, in_maps, list(range(NCORES)), trace=trace)
    if trace and res.exec_time_ns is not None:
        print(f"HW exec time: {res.exec_time_ns} ns")
        _CACHE["exec_time_ns"] = res.exec_time_ns

    y = np.empty((B, D, N), np.float32)
    for core in range(NCORES):
        b, half = core // 2, core % 2
        y[b, :, half * NLOC : (half + 1) * NLOC] = res.results[core]["out"].reshape(
            D, NLOC
        )
    return y


# revision 13
# speedup vs baseline: 1.4534x; 1.4534x over previous
import os
import sys
import types

import numpy as np

sys.path.insert(0, "/opt/trn_rl_repo")

import ml_dtypes  # noqa: E402
import concourse.mybir as mybir  # noqa: E402
import concourse.tile as tile  # noqa: E402
from concourse import bacc  # noqa: E402
from concourse.bass import ts  # noqa: E402
from concourse.bass_utils import run_bass_kernel_spmd  # noqa: E402

BF16 = mybir.dt.bfloat16
F32 = mybir.dt.float32
bfdt = ml_dtypes.bfloat16
AF = mybir.ActivationFunctionType
ALU = mybir.AluOpType

B, D, N = 4, 512, 2048
H, KVH, DH = 8, 2, 64
CONTEXT_LEN = 4096
NLOC = 1024  # tokens per core
P = 128
NCORES = 8
HP = H // 2  # head pairs
NCH = N // P  # 16 key chunks of 128
NCH_LOC = NLOC // P  # 8 local key chunks

_CACHE = {}


def _enable_trace_hook():
    """Register the NTFF profile hook (missing antenv.axon_hooks shim)."""
    try:
        import antenv

        if "antenv.axon_hooks" in sys.modules:
            return
        mod = types.ModuleType("antenv.axon_hooks")

        def set_axon_ntff_profile_hook(h):
            mod._hook = h

        def get_axon_ntff_profile_hook():
            return getattr(mod, "_hook", None)

        mod.set_axon_ntff_profile_hook = set_axon_ntff_profile_hook
        mod.get_axon_ntff_profile_hook = get_axon_ntff_profile_hook
        sys.modules["antenv.axon_hooks"] = mod
        antenv.axon_hooks = mod
        from trn_agent_boot.trn_boot import _ntff_profile_via_ctypes

        set_axon_ntff_profile_hook(_ntff_profile_via_ctypes("/opt/axon/libaxon_pjrt.so"))
    except Exception:
        pass


def _build():
    nc = bacc.Bacc(None, target_bir_lowering=False, debug=False)
    dp = nc.declare_dram_parameter

    x_e = dp("x", [4, P, NLOC], F32, isOutput=False)
    wq_e = dp("wq", [P, 4, 512], BF16, isOutput=False)
    wqs_e = dp("wqs", [P, 4, 512], BF16, isOutput=False)
    wk_e = dp("wk", [P, 4, 128], BF16, isOutput=False)
    wks_e = dp("wks", [P, 4, 128], BF16, isOutput=False)
    wv_e = dp("wv", [P, 4, 128], BF16, isOutput=False)
    wo_e = dp("wo", [P, 4, 512], BF16, isOutput=False)
    cq_e = dp("cq", [P, NLOC], F32, isOutput=False)
    sq_e = dp("sq", [P, NLOC], F32, isOutput=False)
    ck_e = dp("ck", [P, NLOC], F32, isOutput=False)
    sk_e = dp("sk", [P, NLOC], F32, isOutput=False)
    g_e = dp("g", [1, 512], F32, isOutput=False)  # gamma row
    gb_e = dp("gb", [2, 512], F32, isOutput=False)  # [gamma; beta]
    bo_e = dp("bo", [P, 4], F32, isOutput=False)  # bout per (p, chunk)
    ones_e = dp("ones", [P, 1], F32, isOutput=False)
    out_e = dp("out", [4, P, NLOC], F32, isOutput=True)

    with tile.TileContext(nc) as tc:
        with (
            tc.tile_pool(name="persist", bufs=1) as PS,
            tc.tile_pool(name="tmp", bufs=3) as TMP,
            tc.tile_pool(name="exp", bufs=4) as EXPP,
            tc.tile_pool(name="dram", bufs=1, space="DRAM") as DRAM,
        ):
            # ---------------- phase A: inputs -> SBUF ----------------
            x_sb = PS.tile([P, 4, NLOC], F32, name="x")
            for c in range(4):
                nc.sync.dma_start(x_sb[:, c, :], x_e[c])
            ones_sb = PS.tile([P, 1], F32, name="ones")
            nc.sync.dma_start(ones_sb[:], ones_e[:])
            g_sb = PS.tile([1, 512], F32, name="g")
            nc.sync.dma_start(g_sb[:], g_e[:])
            gb_sb = PS.tile([2, 512], F32, name="gb")
            nc.sync.dma_start(gb_sb[:], gb_e[:])
            bo_sb = PS.tile([P, 4], F32, name="bo")
            nc.sync.dma_start(bo_sb[:], bo_e[:])
            wk_sb = PS.tile([P, 4, 128], BF16, name="wk")
            nc.sync.dma_start(wk_sb[:], wk_e[:])
            wks_sb = PS.tile([P, 4, 128], BF16, name="wks")
            nc.sync.dma_start(wks_sb[:], wks_e[:])
            wv_sb = PS.tile([P, 4, 128], BF16, name="wv")
            nc.sync.dma_start(wv_sb[:], wv_e[:])
            ck_sb = PS.tile([P, NLOC], F32, name="ck")
            nc.sync.dma_start(ck_sb[:], ck_e[:])
            sk_sb = PS.tile([P, NLOC], F32, name="sk")
            nc.sync.dma_start(sk_sb[:], sk_e[:])
            wq_sb = PS.tile([P, 4, 512], BF16, name="wq")
            nc.sync.dma_start(wq_sb[:], wq_e[:])
            wqs_sb = PS.tile([P, 4, 512], BF16, name="wqs")
            nc.sync.dma_start(wqs_sb[:], wqs_e[:])
            cq_sb = PS.tile([P, NLOC], F32, name="cq")
            nc.sync.dma_start(cq_sb[:], cq_e[:])
            sq_sb = PS.tile([P, NLOC], F32, name="sq")
            nc.sync.dma_start(sq_sb[:], sq_e[:])
            wo_sb = PS.tile([P, 4, 512], BF16, name="wo")
            nc.sync.dma_start(wo_sb[:], wo_e[:])

            # v lhsT store: slot s = 2*chunk + kvh, [tk, dh+1] with ones col
            v_sb = PS.tile([P, 2 * NCH, DH + 1], BF16, name="vsb")
            nc.gpsimd.memset(v_sb[:, :, DH : DH + 1], 1.0)

            xnb = [PS.tile([P, NLOC], BF16, name=f"xnb{c}") for c in range(4)]
            qr_sb = [PS.tile([P, NLOC], BF16, name=f"qr{i}") for i in range(HP)]
            k_bf = PS.tile([P, NLOC], BF16, name="kbf")
            k_rem = PS.tile([P, NLOC], BF16, name="krem")
            vcp_sb = PS.tile([P, NCH_LOC, 128], BF16, name="vcp")
            ohat = [PS.tile([P, NLOC], BF16, name=f"oh{i}") for i in range(HP)]
            rb_sb = PS.tile([2, NLOC], F32, name="rb")  # [mu*rstd; -1]
            nc.gpsimd.memset(rb_sb[:], -1.0)  # row 0 overwritten with mu*rstd below
            rstd_sb = PS.tile([1, NLOC], F32, name="rstd")
            mu_sb = PS.tile([1, NLOC], F32, name="mu")
            ex2_sb = PS.tile([1, NLOC], F32, name="ex2")
            var_sb = PS.tile([1, NLOC], F32, name="var")
            sd_sb = PS.tile([1, NLOC], F32, name="sd")
            musq_sb = PS.tile([1, NLOC], F32, name="musq")

            ag_in = DRAM.tile([2, P, NLOC], BF16)
            ag_out = DRAM.tile([2, 2, P, NLOC], BF16)

            # ---------------- phase B: layernorm ----------------
            with tc.tile_pool(name="ps_b1", bufs=1, space="PSUM") as PB1:
                stats = PB1.tile([1, 4, 512], F32, name="stats")
                xsq = PS.tile([P, 4, NLOC], F32, name="xsq")
                nc.scalar.activation(
                    xsq[:].rearrange("p a b -> p (a b)"),
                    x_sb[:].rearrange("p a b -> p (a b)"), AF.Square,
                )
                for tq in range(2):
                    for c in range(4):
                        nc.tensor.matmul(
                            stats[:, tq, :], ones_sb[:], x_sb[:, c, ts(tq, 512)],
                            start=(c == 0), stop=(c == 3),
                        )
                for tq in range(2):
                    for c in range(4):
                        nc.tensor.matmul(
                            stats[:, 2 + tq, :], ones_sb[:], xsq[:, c, ts(tq, 512)],
                            start=(c == 0), stop=(c == 3),
                        )
                nc.scalar.mul(mu_sb[:], stats[:, 0:2, :].rearrange("p a b -> p (a b)"), 1.0 / 512)
                nc.scalar.activation(
                    ex2_sb[:], stats[:, 2:4, :].rearrange("p a b -> p (a b)"),
                    AF.Copy, bias=1e-5, scale=1.0 / 512,
                )
                nc.vector.tensor_mul(musq_sb[:], mu_sb[:], mu_sb[:])
                nc.vector.tensor_tensor(var_sb[:], ex2_sb[:], musq_sb[:], ALU.subtract)
                nc.scalar.activation(sd_sb[:], var_sb[:], AF.Sqrt)
                scr = TMP.tile([1, NLOC], F32, tag="den")
                nc.vector.reciprocal_approx_accurate(rstd_sb[:], sd_sb[:], scr[:])
                nc.vector.tensor_mul(rb_sb[0:1, :], mu_sb[:], rstd_sb[:])

            # xn = x * (gamma (x) rstd) - (gamma (x) mu*rstd - beta (x) 1)
            with tc.tile_pool(name="ps_b2", bufs=2, space="PSUM") as PB2:
                for c in range(4):
                    a_ps = PB2.tile([P, 2, 512], F32, tag="aps")
                    b_ps = PB2.tile([P, 2, 512], F32, tag="bps")
                    for tq in range(2):
                        nc.tensor.matmul(
                            a_ps[:, tq, :], g_sb[0:1, ts(c, 128)],
                            rstd_sb[:, ts(tq, 512)], start=True, stop=True,
                        )
                        nc.tensor.matmul(
                            b_ps[:, tq, :], gb_sb[:, ts(c, 128)],
                            rb_sb[:, ts(tq, 512)], start=True, stop=True,
                        )
                    t1 = TMP.tile([P, NLOC], F32, tag="t")
                    nc.vector.tensor_mul(
                        t1[:], x_sb[:, c, :], a_ps[:].rearrange("p a b -> p (a b)")
                    )
                    nc.vector.tensor_tensor(
                        xnb[c][:], t1[:], b_ps[:].rearrange("p a b -> p (a b)"),
                        ALU.subtract,
                    )

            # ---------------- phase C1: k/v projection, rotary, allgather ----------------
            with tc.tile_pool(name="ps_c1", bufs=1, space="PSUM") as PC:
                kp0 = PC.tile([P, 2, 512], F32, name="kp0")
                kp1 = PC.tile([P, 2, 512], F32, name="kp1")
                for sw, (kps, w) in enumerate(((kp0, wk_sb), (kp1, wks_sb))):
                    for tq in range(2):
                        for c in range(4):
                            nc.tensor.matmul(
                                kps[:, tq, :], w[:, c, :], xnb[c][:, ts(tq, 512)],
                                start=(c == 0), stop=(c == 3),
                            )
                t1 = TMP.tile([P, NLOC], F32, tag="t")
                t2 = TMP.tile([P, NLOC], F32, tag="t")
                nc.vector.tensor_mul(t1[:], ck_sb[:], kp0[:].rearrange("p a b -> p (a b)"))
                nc.vector.tensor_mul(t2[:], sk_sb[:], kp1[:].rearrange("p a b -> p (a b)"))
                nc.vector.tensor_add(k_bf[:], t1[:], t2[:])

                for c8 in range(NCH_LOC):
                    vp = PC.tile([P, 128], F32, name=f"vp{c8 % 2}")
                    for c in range(4):
                        nc.tensor.matmul(
                            vp[:], xnb[c][:, ts(c8, 128)], wv_sb[:, c, :],
                            start=(c == 0), stop=(c == 3),
                        )
                    nc.vector.tensor_copy(vcp_sb[:, c8, :], vp[:])

                nc.sync.dma_start(ag_in[0], k_bf[:])
                nc.sync.dma_start(
                    ag_in[1], vcp_sb[:].rearrange("p a b -> p (a b)")
                )
                nc.gpsimd.collective_compute(
                    "AllGather",
                    ALU.bypass,
                    ins=[ag_in[:]],
                    outs=[ag_out[:]],
                    replica_groups=[[0, 1], [2, 3], [4, 5], [6, 7]],
                )

            # local v -> slots 0..15
            nc.vector.tensor_copy(
                v_sb[:, 0 : 2 * NCH_LOC, 0:DH],
                vcp_sb[:].rearrange("p a (g d) -> p (a g) d", g=2),
            )

            # ---------------- phase C2: q projection + rotary ----------------
            with tc.tile_pool(name="ps_c2", bufs=2, space="PSUM") as PC2:
                for i in range(HP):
                    qp0 = PC2.tile([P, 2, 512], F32, tag="qp0")
                    qp1 = PC2.tile([P, 2, 512], F32, tag="qp1")
                    for kps, w in ((qp0, wq_sb), (qp1, wqs_sb)):
                        for tq in range(2):
                            for c in range(4):
                                nc.tensor.matmul(
                                    kps[:, tq, :], w[:, c, ts(i, 128)],
                                    xnb[c][:, ts(tq, 512)],
                                    start=(c == 0), stop=(c == 3),
                                )
                    t1 = TMP.tile([P, NLOC], F32, tag="t")
                    t2 = TMP.tile([P, NLOC], F32, tag="t")
                    nc.vector.tensor_mul(t1[:], cq_sb[:], qp0[:].rearrange("p a b -> p (a b)"))
                    nc.vector.tensor_mul(t2[:], sq_sb[:], qp1[:].rearrange("p a b -> p (a b)"))
                    nc.vector.tensor_add(qr_sb[i][:], t1[:], t2[:])

            # ---------------- remote kv recovery: remote = (ag0 + ag1) - local ----------------
            agk0 = PS.tile([P, NLOC], BF16, name="agk0")
            agk1 = PS.tile([P, NLOC], BF16, name="agk1")
            agv0 = PS.tile([P, NLOC], BF16, name="agv0")
            agv1 = PS.tile([P, NLOC], BF16, name="agv1")
            nc.sync.dma_start(agk0[:], ag_out[0, 0])
            nc.sync.dma_start(agk1[:], ag_out[1, 0])
            nc.sync.dma_start(agv0[:], ag_out[0, 1])
            nc.sync.dma_start(agv1[:], ag_out[1, 1])
            tk = TMP.tile([P, NLOC], F32, tag="t")
            nc.vector.tensor_add(tk[:], agk0[:], agk1[:])
            nc.vector.tensor_tensor(k_rem[:], tk[:], k_bf[:], ALU.subtract)
            tv = TMP.tile([P, NLOC], F32, tag="t")
            nc.vector.tensor_add(tv[:], agv0[:], agv1[:])
            nc.vector.tensor_tensor(
                v_sb[:, 2 * NCH_LOC : 2 * NCH, 0:DH],
                tv[:].rearrange("p (a g d) -> p (a g) d", g=2, d=DH),
                vcp_sb[:].rearrange("p a (g d) -> p (a g) d", g=2),
                ALU.subtract,
            )

            # ---------------- phase D: attention main loop ----------------
            # Software-pipelined: per chunk c emit scores(c) matmuls, then
            # AV(c-1) matmuls (whose exp finished while scores(c) ran), then
            # exp(c). Scores for the GQA head pair run concurrently on
            # disjoint PE row groups (explicit tile_position).
            with (
                tc.tile_pool(name="ps_sc", bufs=2, space="PSUM") as PSC,
                tc.tile_pool(name="ps_av", bufs=2, space="PSUM") as PAV,
            ):
                for hp in range(HP):
                    for tq in range(2):
                        oA = PAV.tile([DH + 1, 512], F32, tag="avA")
                        oB = PAV.tile([DH + 1, 512], F32, tag="avB")
                        prev = None
                        for c in range(NCH + 1):
                            if c < NCH:
                                if c < NCH_LOC:
                                    ksrc = k_bf[:, ts(c, 128)]
                                else:
                                    ksrc = k_rem[:, ts(c - NCH_LOC, 128)]
                                sc = PSC.tile([P, 2, 512], F32, tag="sc")
                                nc.tensor.matmul(
                                    sc[:, 0, :], ksrc[0:64, :],
                                    qr_sb[hp][0:64, ts(tq, 512)],
                                    start=True, stop=True, tile_position=(0, 0),
                                )
                                nc.tensor.matmul(
                                    sc[:, 1, :], ksrc[64:128, :],
                                    qr_sb[hp][64:128, ts(tq, 512)],
                                    start=True, stop=True, tile_position=(64, 0),
                                )
                            if prev is not None:
                                ep, cp = prev
                                nc.tensor.matmul(
                                    oA[:], v_sb[:, 2 * cp + 0, :], ep[:, 0:512],
                                    start=(cp == 0), stop=(cp == NCH - 1),
                                )
                                nc.tensor.matmul(
                                    oB[:], v_sb[:, 2 * cp + 1, :], ep[:, 512:1024],
                                    start=(cp == 0), stop=(cp == NCH - 1),
                                )
                            if c < NCH:
                                e = EXPP.tile([P, NLOC], BF16, tag="e")
                                nc.scalar.activation(
                                    e[:], sc[:].rearrange("p a b -> p (a b)"),
                                    AF.Exp, scale=0.125,
                                )
                                prev = (e, c)
                        # epilogue: divide by denominator row, write o_hat
                        den2 = TMP.tile([1, 1024], F32, tag="den")
                        nc.vector.tensor_copy(den2[0:1, 0:512], oA[DH : DH + 1, :])
                        nc.vector.tensor_copy(den2[0:1, 512:1024], oB[DH : DH + 1, :])
                        rec2 = TMP.tile([1, 1024], F32, tag="den")
                        nc.vector.reciprocal_approx_fast(rec2[:], den2[:])
                        pbA = TMP.tile([64, 512], F32, tag="pb")
                        pbB = TMP.tile([64, 512], F32, tag="pb")
                        nc.gpsimd.partition_broadcast(pbA[:], rec2[0:1, 0:512])
                        nc.gpsimd.partition_broadcast(pbB[:], rec2[0:1, 512:1024])
                        nc.vector.tensor_mul(
                            ohat[hp][0:64, ts(tq, 512)], oA[0:DH, :], pbA[:]
                        )
                        nc.vector.tensor_mul(
                            ohat[hp][64:128, ts(tq, 512)], oB[0:DH, :], pbB[:]
                        )

            # ---------------- phase E: output projection + residual ----------------
            with tc.tile_pool(name="ps_e", bufs=4, space="PSUM") as PE_:
                for mc in range(4):
                    for tq in range(2):
                        yps = PE_.tile([P, 512], F32, tag="yps")
                        for kc in range(4):
                            nc.tensor.matmul(
                                yps[:], wo_sb[:, kc, ts(mc, 128)],
                                ohat[kc][:, ts(tq, 512)],
                                start=(kc == 0), stop=(kc == 3),
                            )
                        yt = TMP.tile([P, 512], F32, tag="yout")
                        nc.vector.tensor_add(yt[:], yps[:], xnb[mc][:, ts(tq, 512)])
                        yo = TMP.tile([P, 512], F32, tag="yout")
                        nc.vector.tensor_scalar_add(yo[:], yt[:], bo_sb[:, mc : mc + 1])
                        nc.sync.dma_start(out_e[mc, :, ts(tq, 512)], yo[:])

    nc.compile()
    return nc


def _host_inputs(x, gamma, beta, Wq, Wkv, Wout, bout):
    """Build the 8 per-core input maps."""
    x = np.asarray(x, np.float32)
    gamma = np.asarray(gamma, np.float32)
    beta = np.asarray(beta, np.float32)
    Wq = np.asarray(Wq, np.float32)
    Wkv = np.asarray(Wkv, np.float32)
    Wout = np.asarray(Wout, np.float32)
    bout = np.asarray(bout, np.float32)

    def swap_heads(W):
        # permute output cols j -> j xor 32 within each 64-block
        c = W.shape[1]
        return np.ascontiguousarray(
            W.reshape(D, c // 64, 2, 32)[:, :, ::-1, :].reshape(D, c)
        )

    def lhsT(W):
        # [D, M] -> [128, 4, M] chunk layout
        return np.ascontiguousarray(
            W.reshape(4, P, W.shape[1]).transpose(1, 0, 2).astype(bfdt)
        )

    Wk = Wkv[:, : KVH * DH]
    Wv = Wkv[:, KVH * DH :]
    wq = lhsT(Wq)
    wqs = lhsT(swap_heads(Wq))
    wk = lhsT(Wk)
    wks = lhsT(swap_heads(Wk))
    wv = lhsT(Wv)
    wo = lhsT(Wout)
    g = np.ascontiguousarray(gamma[None, :])
    gb = np.ascontiguousarray(np.stack([gamma, beta]))
    bo = np.ascontiguousarray(bout.reshape(4, P).T)
    ones = np.ones((P, 1), np.float32)

    # rotary tables (per half)
    j = np.arange(DH)
    inv_freq = 1.0 / (10000.0 ** ((2.0 * (j % 32)) / DH))
    base = ((2.0 * (j % 32)) + 0.4 * DH) / (1.4 * DH)
    sign = np.where(j < 32, -1.0, 1.0)

    tables = []
    for half in range(2):
        pos = half * NLOC + np.arange(NLOC, dtype=np.float64)
        freqs = pos[None, :] * inv_freq[:, None]  # [64, NLOC]
        cos, sin = np.cos(freqs), np.sin(freqs)
        power = (pos - N // 2) / CONTEXT_LEN
        xsc = base[:, None] ** power[None, :]
        cq = np.tile((cos * xsc), (2, 1)).astype(np.float32)
        sq = np.tile((sign[:, None] * sin * xsc), (2, 1)).astype(np.float32)
        ck = np.tile((cos / xsc), (2, 1)).astype(np.float32)
        sk = np.tile((sign[:, None] * sin / xsc), (2, 1)).astype(np.float32)
        tables.append((cq, sq, ck, sk))

    in_maps = []
    for core in range(NCORES):
        b, half = core // 2, core % 2
        xc = np.ascontiguousarray(
            x[b].reshape(4, P, N)[:, :, half * NLOC : (half + 1) * NLOC]
        )
        cq, sq, ck, sk = tables[half]
        in_maps.append(
            {
                "x": xc, "wq": wq, "wqs": wqs, "wk": wk, "wks": wks,
                "wv": wv, "wo": wo, "cq": cq, "sq": sq, "ck": ck, "sk": sk,
                "g": g, "gb": gb, "bo": bo, "ones": ones,
            }
        )
    return in_maps


def kernel(x, gamma, beta, Wq, Wkv, Wout, bout):
    trace = os.environ.get("KERNEL_TRACE", "0") == "1"
    if trace:
        _enable_trace_hook()
    if "nc" not in _CACHE:
        _CACHE["nc"] = _build()
    nc = _CACHE["nc"]
    in_maps = _host_inputs(x, gamma, beta, Wq, Wkv, Wout, bout)
    res = run_bass_kernel_spmd(nc, in_maps, list(range(NCORES)), trace=trace)
    if trace and res.exec_time_ns is not None:
        print(f"HW exec time: {res.exec_time_ns} ns")
        _CACHE["exec_time_ns"] = res.exec_time_ns

    y = np.empty((B, D, N), np.float32)
    for core in range(NCORES):
        b, half = core // 2, core % 2
        y[b, :, half * NLOC : (half + 1) * NLOC] = res.results[core]["out"].reshape(
            D, NLOC
        )
    return y
